# revision 16
# baseline (speedup 1.0000x reference)
"""Multi-head attention (B=2, S=2048, D=1024, H=16, causal mask) on 8 TRN2 cores.

Sharding: core c handles batch b = c//4 and 4 heads g = c%4 (dims 256g..256g+256
of the projection space).  Each core computes a partial output [S, D] (its 4
heads' contribution to the out-projection); the host sums the 4 partials per
batch and adds the output bias.

Device layout (per core) keeps the sequence axis on the SBUF free dimension:
  QT, KT  [256, 2048]  (head-dim on partitions, 2 head-pairs of 128)
  V_aug   16 tiles [128, 4, 65]  (seq on partitions; per head 64 dims + ones col)
  scores  S.T tiles [128 k, 512 q] per head; causal blocks above diagonal skipped
  exp     ScalarE, scale=1/8, mask folded in as a -1e9 bias (one [128,128] tri tile)
  ctx.T   [65, 512] PSUM per (head, q-chunk); row 64 = softmax denominator l
  norm    reciprocal_approx_fast on l, partition_broadcast, DVE multiply
  out     ctxT (4 heads stacked, [256, 2048]) @ o_w slice -> [2048, 1024]

Scheduling notes: emission order sets Tile scheduler priority.  Per q-chunk the
attention pipeline (scores -> exp on ScalarE -> AV) is emitted first; the next
chunk's projections and the previous chunk's out-projection follow so the PE
fills its ACT-bound gaps with them.  A warmup matmul burst + a dummy exp at
priority 0 take the HAM cold-clock ramp and the ACT table load during the
initial DMA window.  ScalarE issues nothing but exp; DMA triggers live on
sync/gpsimd.  PSUM: 2x scores (2 banks each) + 2x proj/outproj (1 bank) +
2x ctx (1 bank) = 8 banks.
"""

import numpy as np
from contextlib import ExitStack

import concourse.bacc as bacc
import concourse.bass as bass
import concourse.tile as tile
from concourse import mybir

P = 128
S = 2048
D = 1024
N_HEADS_TOT = 16
HEADS = 4            # per core
HD = 64
M_DIM = HEADS * HD   # 256
KC = 8               # embed-dim 128-chunks
QCW = 512            # q chunk width
NQC = S // QCW       # 4
NKT = S // P         # 16 k-tiles
F32 = mybir.dt.float32
BF16 = mybir.dt.bfloat16
EXPF = mybir.ActivationFunctionType.Exp
NEG = -1.0e9

TRACE = False
LAST_RESULTS = None
_NC_CACHE = {}

# x column groups loaded as single DMAs: (col0, width); chunk n -> (group, off)
XGRP = ((0, QCW), (QCW, QCW), (2 * QCW, 2 * QCW))
CHUNK_XG = {0: (0, 0), 1: (1, 0), 2: (2, 0), 3: (2, QCW)}


def build_nc(mode: str, compile_: bool = True, has_bias: bool = False,
             probes: bool = False) -> bass.Bass:
    """mode in {causal, nomask, generic}"""
    nc = bacc.Bacc("TRN2", target_bir_lowering=False, debug=False)
    prb = {}
    if probes:
        for nm, shape in (("p_qt", [P, S]), ("p_kt", [P, S]),
                          ("p_va", [P, HEADS * (HD + 1)]), ("p_ct", [P, S])):
            prb[nm] = nc.dram_tensor(nm, shape, F32, kind="ExternalOutput").ap()
    # x/w/ow arrive pre-arranged by the host as contiguous SBUF images so each
    # load is one linear DMA (x: per column group g, [128, KC*w_g] flattened)
    xq = nc.dram_tensor("xqP", [P, KC * S], BF16, kind="ExternalInput").ap()
    xk = nc.dram_tensor("xkP", [P, KC * S], BF16, kind="ExternalInput").ap()
    xv = nc.dram_tensor("xvP", [P, KC * S], BF16, kind="ExternalInput").ap()
    wq = nc.dram_tensor("wqP", [P, KC * M_DIM], BF16, kind="ExternalInput").ap()
    wk = nc.dram_tensor("wkP", [P, KC * M_DIM], BF16, kind="ExternalInput").ap()
    wv = nc.dram_tensor("wvP", [P, KC * M_DIM], BF16, kind="ExternalInput").ap()
    ow = nc.dram_tensor("owP", [P, 2 * D], BF16, kind="ExternalInput").ap()
    btri = nc.dram_tensor("btri", [P, P], F32, kind="ExternalInput").ap()
    wbias = None
    if has_bias:
        wbias = nc.dram_tensor("wbias", [3, M_DIM], BF16, kind="ExternalInput").ap()
    bfull = None
    if mode == "generic":
        bfull = nc.dram_tensor("biasT", [S, S], F32, kind="ExternalInput").ap()
    out = nc.dram_tensor("out", [S, D], BF16, kind="ExternalOutput").ap()

    xmaps = {"q": xq, "k": xk, "v": xv}

    with tile.TileContext(nc) as tc, ExitStack() as ctx:
        consts = ctx.enter_context(tc.tile_pool(name="consts", bufs=1))
        xpool = ctx.enter_context(tc.tile_pool(name="xpool", bufs=1))
        qkv = ctx.enter_context(tc.tile_pool(name="qkv", bufs=1))
        ppool = ctx.enter_context(tc.tile_pool(name="ppool", bufs=6))
        bpool = ctx.enter_context(tc.tile_pool(name="bpool", bufs=2))
        small = ctx.enter_context(tc.tile_pool(name="small", bufs=2))
        outp = ctx.enter_context(tc.tile_pool(name="outp", bufs=2))
        scp = ctx.enter_context(tc.tile_pool(name="spsum", bufs=2, space="PSUM"))
        pjp = ctx.enter_context(tc.tile_pool(name="pjpsum", bufs=2, space="PSUM"))
        cpp = ctx.enter_context(tc.tile_pool(name="cpsum", bufs=2, space="PSUM"))

        # ---- PE warmup + ACT table preload (runs during initial DMA wait) ----
        warm_sb = consts.tile([P, QCW], BF16, name="warm_sb")
        nc.vector.memset(warm_sb, 0.0)
        warm_ps = scp.tile([P, 2, QCW], F32, name="s_ps")
        for i in range(10):
            nc.tensor.matmul(warm_ps[:, 0, :], lhsT=warm_sb[:, 0:P],
                             rhs=warm_sb, start=(i == 0), stop=(i == 9))
        warm_rd = consts.tile([1, 8], F32, name="warm_rd")
        nc.vector.tensor_copy(warm_rd, warm_ps[0:1, 0, 0:8])
        warm_ex = consts.tile([P, 8], BF16, name="warm_ex")
        nc.scalar.activation(warm_ex, warm_sb[:, 0:8], EXPF, scale=0.125)

        # ---- resident weights + x loads: contiguous-image DMAs, q/k/v order
        # interleaved so attention-0's inputs land earliest ----
        w_sb, w_aug, x_t = {}, {}, {}
        x_aug = None

        def load_w(key, ap_dram):
            t = consts.tile([P, KC, M_DIM], BF16, name=f"w{key}")
            nc.sync.dma_start(
                out=t, in_=ap_dram.rearrange("p (kc m) -> p kc m", kc=KC))
            w_sb[key] = t

        def load_x(key, g):
            c0, cw = XGRP[g]
            t = xpool.tile([P, KC, cw], BF16, name=f"x{key}{g}")
            nc.sync.dma_start(
                out=t,
                in_=xmaps[key][:, KC * c0:KC * (c0 + cw)]
                .rearrange("p (kc q) -> p kc q", kc=KC))
            x_t[(key, g)] = t

        load_w("q", wq)
        load_x("q", 0)
        load_w("k", wk)
        load_x("k", 0)
        btri_sb = consts.tile([P, P], F32, name="btri_sb")
        nc.sync.dma_start(out=btri_sb, in_=btri)
        load_w("v", wv)
        load_x("v", 0)
        for key in ("q", "k", "v"):
            load_x(key, 1)
        ow_sb = consts.tile([P, 2, D], BF16, name="ow_sb")
        nc.sync.dma_start(
            out=ow_sb, in_=ow.rearrange("p (pr d) -> p pr d", pr=2))
        for key in ("q", "k", "v"):
            load_x(key, 2)
        if has_bias:
            for i, key in enumerate(("q", "k", "v")):
                a = consts.tile([1, M_DIM], BF16, name=f"w{key}_aug")
                nc.sync.dma_start(out=a, in_=wbias[i:i + 1, :])
                w_aug[key] = a
            x_aug = consts.tile([1, S], BF16, name="x_aug")
            nc.vector.memset(x_aug, 1.0)

        QT = [qkv.tile([P, S], BF16, name=f"QT{pr}") for pr in range(2)]
        KT = [qkv.tile([P, S], BF16, name=f"KT{pr}") for pr in range(2)]
        CT = [qkv.tile([P, S], BF16, name=f"CT{pr}") for pr in range(2)]
        VA = [qkv.tile([P, HEADS, HD + 1], BF16, name=f"VA{t}") for t in range(NKT)]
        # the softmax-denominator ones column of each V tile never changes
        for m in range(NKT):
            nc.gpsimd.memset(VA[m][:, :, HD:HD + 1], 1.0)

        def proj_stage(n):
            g, off = CHUNK_XG[n]
            for key, dest in (("q", QT), ("k", KT)):
                for m in range(2):
                    ps = pjp.tile([P, QCW], F32, name="p_ps")
                    for kc in range(KC):
                        nc.tensor.matmul(
                            ps,
                            lhsT=w_sb[key][:, kc, P * m:P * (m + 1)],
                            rhs=x_t[(key, g)][:, kc, off:off + QCW],
                            start=(kc == 0),
                            stop=(not has_bias and kc == KC - 1))
                    if has_bias:
                        nc.tensor.matmul(
                            ps,
                            lhsT=w_aug[key][0:1, P * m:P * (m + 1)],
                            rhs=x_aug[0:1, QCW * n:QCW * (n + 1)],
                            start=False, stop=True)
                    nc.vector.tensor_copy(
                        dest[m][:, QCW * n:QCW * (n + 1)], ps)
            for mv in range(4):
                m = 4 * n + mv
                ps = pjp.tile([P, QCW], F32, name="p_ps")
                for kc in range(KC):
                    nc.tensor.matmul(
                        ps[:, 0:M_DIM],
                        lhsT=x_t[("v", g)][:, kc, off + P * mv:off + P * (mv + 1)],
                        rhs=w_sb["v"][:, kc, :],
                        start=(kc == 0),
                        stop=(not has_bias and kc == KC - 1))
                if has_bias:
                    nc.tensor.matmul(
                        ps[:, 0:M_DIM],
                        lhsT=x_aug[0:1, QCW * n + P * mv:QCW * n + P * (mv + 1)],
                        rhs=w_aug["v"],
                        start=False, stop=True)
                nc.vector.tensor_copy(
                    VA[m][:, :, 0:HD],
                    ps[:, 0:M_DIM].rearrange("p (h d) -> p h d", h=HEADS))

        def outproj_tile(qc, mq, final=False):
            out_sb = outp.tile([P, D], BF16, name="out_sb")
            q0 = QCW * qc + P * mq
            for ne in range(2):
                o_ps = pjp.tile([P, QCW], F32, name="p_ps")
                for pr2 in range(2):
                    nc.tensor.matmul(
                        o_ps,
                        lhsT=CT[pr2][:, q0:q0 + P],
                        rhs=ow_sb[:, pr2, QCW * ne:QCW * (ne + 1)],
                        start=(pr2 == 0), stop=(pr2 == 1))
                # in the drained tail ScalarE is idle: split the two PSUM
                # casts across DVE and ACT so they run in parallel
                if final and ne == 1:
                    nc.scalar.copy(out_sb[:, QCW * ne:QCW * (ne + 1)], o_ps)
                else:
                    nc.vector.tensor_copy(out_sb[:, QCW * ne:QCW * (ne + 1)], o_ps)
            nc.gpsimd.dma_start(out=out[q0:q0 + P, :], in_=out_sb)

        def outproj_groups(qc):
            return [(lambda mq=mq: outproj_tile(qc, mq)) for mq in range(QCW // P)]

        def attention(qc, pr, fillers=None):
            nt = 4 * qc + 4 if mode == "causal" else NKT
            ctxs = [cpp.tile([HD + 1, QCW], F32, name="ctx_ps")
                    for _ in range(2)]
            queues = ([], [])
            fillers = list(fillers or [])

            def flush_ctx(j):
                t0, p0, o0 = queues[j].pop(0)
                nc.tensor.matmul(
                    ctxs[j][:, o0:],
                    lhsT=VA[t0][:, 2 * pr + j, :],
                    rhs=p0[:, j, o0:],
                    start=(t0 == 0), stop=(t0 == nt - 1),
                    skip_group_check=True)

            for t in range(nt):
                o = max(0, P * t - QCW * qc) if mode == "causal" else 0
                s_ps = scp.tile([P, 2, QCW], F32, name="s_ps")
                for j in range(2):
                    nc.tensor.matmul(
                        s_ps[:, j, o:],
                        lhsT=KT[pr][HD * j:HD * (j + 1), P * t:P * (t + 1)],
                        rhs=QT[pr][HD * j:HD * (j + 1),
                                   QCW * qc + o:QCW * (qc + 1)],
                        start=True, stop=True,
                        tile_position=(HD * j, 0))
                if mode == "causal" and t >= 4 * qc:
                    nc.vector.tensor_add(
                        s_ps[:, :, o:o + P],
                        s_ps[:, :, o:o + P],
                        btri_sb.rearrange("p (a q) -> p a q", a=1)
                        .to_broadcast([P, 2, P]))
                elif mode == "generic":
                    bt = bpool.tile([P, QCW], F32, name="bt")
                    nc.sync.dma_start(
                        out=bt,
                        in_=bfull[P * t:P * (t + 1), QCW * qc:QCW * (qc + 1)])
                    nc.vector.tensor_add(
                        s_ps, s_ps,
                        bt.rearrange("p (a q) -> p a q", a=1)
                        .to_broadcast([P, 2, QCW]))
                p_sb = ppool.tile([P, 2, QCW], BF16, name="p_sb")
                nc.scalar.activation(
                    p_sb[:, :, o:], s_ps[:, :, o:], EXPF, scale=0.125)
                for j in range(2):
                    queues[j].append((t, p_sb, o))
                for j in range(2):
                    if len(queues[j]) > 2:
                        flush_ctx(j)
                if fillers and t % 4 == 3:
                    fillers.pop(0)()
            for j in range(2):
                while queues[j]:
                    flush_ctx(j)
            while fillers:
                fillers.pop(0)()
            # normalize: reciprocal of denominator row, broadcast, scale ctx
            # (partition_broadcast must target base partition 0 — HW ucode
            # ignores an output partition offset, so one fresh tile per head)
            for j in range(2):
                l_sb = small.tile([1, QCW], F32, name="l_sb", bufs=4)
                nc.vector.tensor_copy(l_sb, ctxs[j][HD:HD + 1, :])
                r_sb = small.tile([1, QCW], F32, name="r_sb", bufs=4)
                nc.vector.reciprocal_approx_fast(out=r_sb, in_=l_sb)
                rbc = small.tile([HD, QCW], F32, name="rbc", bufs=2)
                nc.gpsimd.partition_broadcast(out_ap=rbc, in_ap=r_sb)
                nc.vector.tensor_mul(
                    CT[pr][HD * j:HD * (j + 1), QCW * qc:QCW * (qc + 1)],
                    ctxs[j][0:HD, :], rbc)

        proj_stage(0)
        for qc in range(NQC):
            og = outproj_groups(qc - 1) if qc >= 1 else []
            attention(qc, 0, fillers=og[:2])
            attention(qc, 1, fillers=og[2:])
            if qc + 1 < NQC:
                proj_stage(qc + 1)
        # keep the PE HAM-warm through the final normalize so the last
        # out-projection runs at full clock
        tw_ps = scp.tile([P, 2, QCW], F32, name="s_ps")
        for i in range(12):
            nc.tensor.matmul(tw_ps[:, 0, 0:256], lhsT=warm_sb[:, 0:P],
                             rhs=warm_sb[:, 0:256], start=(i == 0), stop=(i == 11))
        nc.vector.tensor_copy(warm_rd, tw_ps[0:1, 0, 0:8])
        for mq in range(QCW // P):
            outproj_tile(NQC - 1, mq, final=True)
        if probes:
            nc.sync.dma_start(out=prb["p_qt"].bitcast(BF16)[:, 0:S], in_=QT[0])
            nc.sync.dma_start(out=prb["p_kt"].bitcast(BF16)[:, 0:S], in_=KT[0])
            nc.sync.dma_start(
                out=prb["p_va"].bitcast(BF16)[:, 0:HEADS * (HD + 1)],
                in_=VA[0].rearrange("p h d -> p (h d)"))
            nc.sync.dma_start(out=prb["p_ct"].bitcast(BF16)[:, 0:S], in_=CT[0])

    if compile_:
        nc.compile()
    return nc


def _get_nc(mode, has_bias):
    key = (mode, has_bias)
    if key not in _NC_CACHE:
        _NC_CACHE[key] = build_nc(mode, has_bias=has_bias)
    return _NC_CACHE[key]


def _tri_bias():
    g = np.arange(P, dtype=np.int64)
    return np.where(g[None, :] < g[:, None], np.float32(NEG), np.float32(0.0))


def host_prep(query, key, value, attn_mask, q_w, q_b, k_w, k_b, v_w, v_b, o_w, o_b):
    """Build (mode, in_maps) for the 8 cores."""
    mask = np.asarray(attn_mask).astype(bool)
    if np.array_equal(mask, np.triu(np.ones((S, S), bool), 1)):
        mode = "causal"
    elif not mask.any():
        mode = "nomask"
    else:
        mode = "generic"

    import ml_dtypes
    bf16 = ml_dtypes.bfloat16

    def img(xT, kc=KC):
        """[kc*128, W] -> SBUF image [128, kc*W] (kc-major per partition)."""
        r, w = xT.shape
        return np.ascontiguousarray(
            xT.reshape(kc, P, w).transpose(1, 0, 2).reshape(P, kc * w))

    def prep_x(x):
        xT = np.ascontiguousarray(np.asarray(x).T).astype(bf16)
        return np.concatenate(
            [img(xT[:, c0:c0 + cw]) for c0, cw in XGRP], axis=1)

    xs = {}
    for b in range(2):
        xs[b] = (prep_x(query[b]), prep_x(key[b]), prep_x(value[b]))

    tri = _tri_bias()
    biasT = None
    if mode == "generic":
        biasT = np.ascontiguousarray(
            np.where(mask, np.float32(NEG), np.float32(0.0)).T)
    has_bias = any(np.asarray(v).any() for v in (q_b, k_b, v_b))

    def prep_w(w, sl):
        return img(np.ascontiguousarray(np.asarray(w)[sl].T).astype(bf16))

    in_maps = []
    for c in range(8):
        b, g = divmod(c, 4)
        sl = slice(M_DIM * g, M_DIM * (g + 1))
        m = {
            "xqP": xs[b][0], "xkP": xs[b][1], "xvP": xs[b][2],
            "wqP": prep_w(q_w, sl),
            "wkP": prep_w(k_w, sl),
            "wvP": prep_w(v_w, sl),
            "owP": img(np.ascontiguousarray(
                np.asarray(o_w)[:, sl].T).astype(bf16), kc=2),
            "btri": tri,
        }
        if has_bias:
            m["wbias"] = np.stack([
                np.asarray(v)[sl].astype(bf16) for v in (q_b, k_b, v_b)])
        if mode == "generic":
            m["biasT"] = biasT
        in_maps.append(m)
    return mode, in_maps


def kernel(**inputs) -> np.ndarray:
    global LAST_RESULTS
    from concourse.bass_utils import run_bass_kernel_spmd

    mode, in_maps = host_prep(**inputs)
    has_bias = any(
        np.asarray(inputs[k]).any() for k in ("q_b", "k_b", "v_b"))
    nc = _get_nc(mode, has_bias)
    res = run_bass_kernel_spmd(nc, in_maps, core_ids=list(range(8)), trace=TRACE)
    LAST_RESULTS = res
    parts = [np.asarray(res.results[c]["out"]).astype(np.float32)
             for c in range(8)]
    o_b = np.asarray(inputs["o_b"]).astype(np.float32)
    out = np.stack([
        parts[0] + parts[1] + parts[2] + parts[3],
        parts[4] + parts[5] + parts[6] + parts[7],
    ], axis=0) + o_b[None, None, :]
    return out.astype(np.float32)


# revision 17
# speedup vs baseline: 1.0143x; 1.0143x over previous
"""Multi-head attention (B=2, S=2048, D=1024, H=16, causal mask) on 8 TRN2 cores.

Sharding: core c handles batch b = c//4 and 4 heads g = c%4 (dims 256g..256g+256
of the projection space).  Each core computes a partial output [S, D] (its 4
heads' contribution to the out-projection); the host sums the 4 partials per
batch and adds the output bias.

Device layout (per core) keeps the sequence axis on the SBUF free dimension:
  QT, KT  [256, 2048]  (head-dim on partitions, 2 head-pairs of 128)
  V_aug   16 tiles [128, 4, 65]  (seq on partitions; per head 64 dims + ones col)
  scores  S.T tiles [128 k, 512 q] per head; causal blocks above diagonal skipped
  exp     ScalarE, scale=1/8, mask folded in as a -1e9 bias (one [128,128] tri tile)
  ctx.T   [65, 512] PSUM per (head, q-chunk); row 64 = softmax denominator l
  norm    reciprocal_approx_fast on l, partition_broadcast, DVE multiply
  out     ctxT (4 heads stacked, [256, 2048]) @ o_w slice -> [2048, 1024]

Scheduling notes: emission order sets Tile scheduler priority.  Per q-chunk the
attention pipeline (scores -> exp on ScalarE -> AV) is emitted first; the next
chunk's projections and the previous chunk's out-projection follow so the PE
fills its ACT-bound gaps with them.  A warmup matmul burst + a dummy exp at
priority 0 take the HAM cold-clock ramp and the ACT table load during the
initial DMA window.  ScalarE issues nothing but exp; DMA triggers live on
sync/gpsimd.  PSUM: 2x scores (2 banks each) + 2x proj/outproj (1 bank) +
2x ctx (1 bank) = 8 banks.
"""

import numpy as np
from contextlib import ExitStack

import concourse.bacc as bacc
import concourse.bass as bass
import concourse.tile as tile
from concourse import mybir

P = 128
S = 2048
D = 1024
N_HEADS_TOT = 16
HEADS = 4            # per core
HD = 64
M_DIM = HEADS * HD   # 256
KC = 8               # embed-dim 128-chunks
QCW = 512            # q chunk width
NQC = S // QCW       # 4
NKT = S // P         # 16 k-tiles
F32 = mybir.dt.float32
BF16 = mybir.dt.bfloat16
EXPF = mybir.ActivationFunctionType.Exp
NEG = -1.0e9

TRACE = False
LAST_RESULTS = None
_NC_CACHE = {}

# x column groups loaded as single DMAs: (col0, width); chunk n -> (group, off)
XGRP = ((0, QCW), (QCW, QCW), (2 * QCW, 2 * QCW))
CHUNK_XG = {0: (0, 0), 1: (1, 0), 2: (2, 0), 3: (2, QCW)}


def build_nc(mode: str, compile_: bool = True, has_bias: bool = False,
             probes: bool = False) -> bass.Bass:
    """mode in {causal, nomask, generic}"""
    nc = bacc.Bacc("TRN2", target_bir_lowering=False, debug=False)
    prb = {}
    if probes:
        for nm, shape in (("p_qt", [P, S]), ("p_kt", [P, S]),
                          ("p_va", [P, HEADS * (HD + 1)]), ("p_ct", [P, S])):
            prb[nm] = nc.dram_tensor(nm, shape, F32, kind="ExternalOutput").ap()
    # x/w/ow arrive pre-arranged by the host as contiguous SBUF images so each
    # load is one linear DMA (x: per column group g, [128, KC*w_g] flattened)
    xq = nc.dram_tensor("xqP", [P, KC * S], BF16, kind="ExternalInput").ap()
    xk = nc.dram_tensor("xkP", [P, KC * S], BF16, kind="ExternalInput").ap()
    xv = nc.dram_tensor("xvP", [P, KC * S], BF16, kind="ExternalInput").ap()
    wq = nc.dram_tensor("wqP", [P, KC * M_DIM], BF16, kind="ExternalInput").ap()
    wk = nc.dram_tensor("wkP", [P, KC * M_DIM], BF16, kind="ExternalInput").ap()
    wv = nc.dram_tensor("wvP", [P, KC * M_DIM], BF16, kind="ExternalInput").ap()
    ow = nc.dram_tensor("owP", [P, 2 * D], BF16, kind="ExternalInput").ap()
    btri = nc.dram_tensor("btri", [P, P], F32, kind="ExternalInput").ap()
    wbias = None
    if has_bias:
        wbias = nc.dram_tensor("wbias", [3, M_DIM], BF16, kind="ExternalInput").ap()
    bfull = None
    if mode == "generic":
        bfull = nc.dram_tensor("biasT", [S, S], F32, kind="ExternalInput").ap()
    out = nc.dram_tensor("out", [S, D], BF16, kind="ExternalOutput").ap()

    xmaps = {"q": xq, "k": xk, "v": xv}

    with tile.TileContext(nc) as tc, ExitStack() as ctx:
        consts = ctx.enter_context(tc.tile_pool(name="consts", bufs=1))
        xpool = ctx.enter_context(tc.tile_pool(name="xpool", bufs=1))
        qkv = ctx.enter_context(tc.tile_pool(name="qkv", bufs=1))
        ppool = ctx.enter_context(tc.tile_pool(name="ppool", bufs=6))
        bpool = ctx.enter_context(tc.tile_pool(name="bpool", bufs=2))
        small = ctx.enter_context(tc.tile_pool(name="small", bufs=2))
        outp = ctx.enter_context(tc.tile_pool(name="outp", bufs=2))
        scp = ctx.enter_context(tc.tile_pool(name="spsum", bufs=2, space="PSUM"))
        pjp = ctx.enter_context(tc.tile_pool(name="pjpsum", bufs=2, space="PSUM"))
        cpp = ctx.enter_context(tc.tile_pool(name="cpsum", bufs=2, space="PSUM"))

        # ---- PE warmup + ACT table preload (runs during initial DMA wait) ----
        warm_sb = consts.tile([P, QCW], BF16, name="warm_sb")
        nc.vector.memset(warm_sb, 0.0)
        warm_ps = scp.tile([P, 2, QCW], F32, name="s_ps")
        for i in range(10):
            nc.tensor.matmul(warm_ps[:, 0, :], lhsT=warm_sb[:, 0:P],
                             rhs=warm_sb, start=(i == 0), stop=(i == 9))
        warm_rd = consts.tile([1, 8], F32, name="warm_rd")
        nc.vector.tensor_copy(warm_rd, warm_ps[0:1, 0, 0:8])
        warm_ex = consts.tile([P, 8], BF16, name="warm_ex")
        nc.scalar.activation(warm_ex, warm_sb[:, 0:8], EXPF, scale=0.125)

        # ---- resident weights + x loads: contiguous-image DMAs, q/k/v order
        # interleaved so attention-0's inputs land earliest ----
        w_sb, w_aug, x_t = {}, {}, {}
        x_aug = None

        def load_w(key, ap_dram):
            t = consts.tile([P, KC, M_DIM], BF16, name=f"w{key}")
            nc.sync.dma_start(
                out=t, in_=ap_dram.rearrange("p (kc m) -> p kc m", kc=KC))
            w_sb[key] = t

        def load_x(key, g):
            c0, cw = XGRP[g]
            t = xpool.tile([P, KC, cw], BF16, name=f"x{key}{g}")
            nc.sync.dma_start(
                out=t,
                in_=xmaps[key][:, KC * c0:KC * (c0 + cw)]
                .rearrange("p (kc q) -> p kc q", kc=KC))
            x_t[(key, g)] = t

        load_w("q", wq)
        load_x("q", 0)
        load_w("k", wk)
        load_x("k", 0)
        btri_sb = consts.tile([P, P], F32, name="btri_sb")
        nc.sync.dma_start(out=btri_sb, in_=btri)
        load_w("v", wv)
        load_x("v", 0)
        for key in ("q", "k", "v"):
            load_x(key, 1)
        ow_sb = consts.tile([P, 2, D], BF16, name="ow_sb")
        nc.sync.dma_start(
            out=ow_sb, in_=ow.rearrange("p (pr d) -> p pr d", pr=2))
        for key in ("q", "k", "v"):
            load_x(key, 2)
        if has_bias:
            for i, key in enumerate(("q", "k", "v")):
                a = consts.tile([1, M_DIM], BF16, name=f"w{key}_aug")
                nc.sync.dma_start(out=a, in_=wbias[i:i + 1, :])
                w_aug[key] = a
            x_aug = consts.tile([1, S], BF16, name="x_aug")
            nc.vector.memset(x_aug, 1.0)

        QT = [qkv.tile([P, S], BF16, name=f"QT{pr}") for pr in range(2)]
        KT = [qkv.tile([P, S], BF16, name=f"KT{pr}") for pr in range(2)]
        CT = [qkv.tile([P, S], BF16, name=f"CT{pr}") for pr in range(2)]
        VA = [qkv.tile([P, HEADS, HD + 1], BF16, name=f"VA{t}") for t in range(NKT)]
        # the softmax-denominator ones column of each V tile never changes
        for m in range(NKT):
            nc.gpsimd.memset(VA[m][:, :, HD:HD + 1], 1.0)

        def proj_stage(n):
            g, off = CHUNK_XG[n]
            for key, dest in (("q", QT), ("k", KT)):
                for m in range(2):
                    ps = pjp.tile([P, QCW], F32, name="p_ps")
                    for kc in range(KC):
                        nc.tensor.matmul(
                            ps,
                            lhsT=w_sb[key][:, kc, P * m:P * (m + 1)],
                            rhs=x_t[(key, g)][:, kc, off:off + QCW],
                            start=(kc == 0),
                            stop=(not has_bias and kc == KC - 1))
                    if has_bias:
                        nc.tensor.matmul(
                            ps,
                            lhsT=w_aug[key][0:1, P * m:P * (m + 1)],
                            rhs=x_aug[0:1, QCW * n:QCW * (n + 1)],
                            start=False, stop=True)
                    nc.vector.tensor_copy(
                        dest[m][:, QCW * n:QCW * (n + 1)], ps)
            for mv in range(4):
                m = 4 * n + mv
                ps = pjp.tile([P, QCW], F32, name="p_ps")
                for kc in range(KC):
                    nc.tensor.matmul(
                        ps[:, 0:M_DIM],
                        lhsT=x_t[("v", g)][:, kc, off + P * mv:off + P * (mv + 1)],
                        rhs=w_sb["v"][:, kc, :],
                        start=(kc == 0),
                        stop=(not has_bias and kc == KC - 1))
                if has_bias:
                    nc.tensor.matmul(
                        ps[:, 0:M_DIM],
                        lhsT=x_aug[0:1, QCW * n + P * mv:QCW * n + P * (mv + 1)],
                        rhs=w_aug["v"],
                        start=False, stop=True)
                nc.vector.tensor_copy(
                    VA[m][:, :, 0:HD],
                    ps[:, 0:M_DIM].rearrange("p (h d) -> p h d", h=HEADS))

        def outproj_tile(qc, mq, final=False):
            out_sb = outp.tile([P, D], BF16, name="out_sb")
            q0 = QCW * qc + P * mq
            for ne in range(2):
                o_ps = pjp.tile([P, QCW], F32, name="p_ps")
                for pr2 in range(2):
                    nc.tensor.matmul(
                        o_ps,
                        lhsT=CT[pr2][:, q0:q0 + P],
                        rhs=ow_sb[:, pr2, QCW * ne:QCW * (ne + 1)],
                        start=(pr2 == 0), stop=(pr2 == 1))
                # (keep these on DVE: ScalarE copies would force an ACT
                # table swap away from the Exp set, ~2.7us each way)
                nc.vector.tensor_copy(out_sb[:, QCW * ne:QCW * (ne + 1)], o_ps)
            nc.gpsimd.dma_start(out=out[q0:q0 + P, :], in_=out_sb)

        def outproj_groups(qc):
            return [(lambda mq=mq: outproj_tile(qc, mq)) for mq in range(QCW // P)]

        def attention(qc, pr, fillers=None):
            nt = 4 * qc + 4 if mode == "causal" else NKT
            ctxs = [cpp.tile([HD + 1, QCW], F32, name="ctx_ps")
                    for _ in range(2)]
            queues = ([], [])
            fillers = list(fillers or [])

            def flush_ctx(j):
                t0, p0, o0 = queues[j].pop(0)
                nc.tensor.matmul(
                    ctxs[j][:, o0:],
                    lhsT=VA[t0][:, 2 * pr + j, :],
                    rhs=p0[:, j, o0:],
                    start=(t0 == 0), stop=(t0 == nt - 1),
                    skip_group_check=True)

            for t in range(nt):
                o = max(0, P * t - QCW * qc) if mode == "causal" else 0
                s_ps = scp.tile([P, 2, QCW], F32, name="s_ps")
                for j in range(2):
                    nc.tensor.matmul(
                        s_ps[:, j, o:],
                        lhsT=KT[pr][HD * j:HD * (j + 1), P * t:P * (t + 1)],
                        rhs=QT[pr][HD * j:HD * (j + 1),
                                   QCW * qc + o:QCW * (qc + 1)],
                        start=True, stop=True,
                        tile_position=(HD * j, 0))
                if mode == "causal" and t >= 4 * qc:
                    nc.vector.tensor_add(
                        s_ps[:, :, o:o + P],
                        s_ps[:, :, o:o + P],
                        btri_sb.rearrange("p (a q) -> p a q", a=1)
                        .to_broadcast([P, 2, P]))
                elif mode == "generic":
                    bt = bpool.tile([P, QCW], F32, name="bt")
                    nc.sync.dma_start(
                        out=bt,
                        in_=bfull[P * t:P * (t + 1), QCW * qc:QCW * (qc + 1)])
                    nc.vector.tensor_add(
                        s_ps, s_ps,
                        bt.rearrange("p (a q) -> p a q", a=1)
                        .to_broadcast([P, 2, QCW]))
                p_sb = ppool.tile([P, 2, QCW], BF16, name="p_sb")
                nc.scalar.activation(
                    p_sb[:, :, o:], s_ps[:, :, o:], EXPF, scale=0.125)
                for j in range(2):
                    queues[j].append((t, p_sb, o))
                for j in range(2):
                    if len(queues[j]) > 2:
                        flush_ctx(j)
                if fillers and t % 4 == 3:
                    fillers.pop(0)()
            for j in range(2):
                while queues[j]:
                    flush_ctx(j)
            while fillers:
                fillers.pop(0)()
            # normalize: reciprocal of denominator row, broadcast, scale ctx
            # (partition_broadcast must target base partition 0 — HW ucode
            # ignores an output partition offset, so one fresh tile per head)
            for j in range(2):
                l_sb = small.tile([1, QCW], F32, name="l_sb", bufs=4)
                nc.vector.tensor_copy(l_sb, ctxs[j][HD:HD + 1, :])
                r_sb = small.tile([1, QCW], F32, name="r_sb", bufs=4)
                nc.vector.reciprocal_approx_fast(out=r_sb, in_=l_sb)
                rbc = small.tile([HD, QCW], F32, name="rbc", bufs=2)
                nc.gpsimd.partition_broadcast(out_ap=rbc, in_ap=r_sb)
                nc.vector.tensor_mul(
                    CT[pr][HD * j:HD * (j + 1), QCW * qc:QCW * (qc + 1)],
                    ctxs[j][0:HD, :], rbc)

        proj_stage(0)
        for qc in range(NQC):
            og = outproj_groups(qc - 1) if qc >= 1 else []
            attention(qc, 0, fillers=og[:2])
            attention(qc, 1, fillers=og[2:])
            if qc + 1 < NQC:
                proj_stage(qc + 1)
        # keep the PE HAM-warm through the final normalize so the last
        # out-projection runs at full clock
        tw_ps = scp.tile([P, 2, QCW], F32, name="s_ps")
        for i in range(12):
            nc.tensor.matmul(tw_ps[:, 0, 0:256], lhsT=warm_sb[:, 0:P],
                             rhs=warm_sb[:, 0:256], start=(i == 0), stop=(i == 11))
        nc.vector.tensor_copy(warm_rd, tw_ps[0:1, 0, 0:8])
        for mq in range(QCW // P):
            outproj_tile(NQC - 1, mq, final=True)
        if probes:
            nc.sync.dma_start(out=prb["p_qt"].bitcast(BF16)[:, 0:S], in_=QT[0])
            nc.sync.dma_start(out=prb["p_kt"].bitcast(BF16)[:, 0:S], in_=KT[0])
            nc.sync.dma_start(
                out=prb["p_va"].bitcast(BF16)[:, 0:HEADS * (HD + 1)],
                in_=VA[0].rearrange("p h d -> p (h d)"))
            nc.sync.dma_start(out=prb["p_ct"].bitcast(BF16)[:, 0:S], in_=CT[0])

    if compile_:
        nc.compile()
    return nc


def _get_nc(mode, has_bias):
    key = (mode, has_bias)
    if key not in _NC_CACHE:
        _NC_CACHE[key] = build_nc(mode, has_bias=has_bias)
    return _NC_CACHE[key]


def _tri_bias():
    g = np.arange(P, dtype=np.int64)
    return np.where(g[None, :] < g[:, None], np.float32(NEG), np.float32(0.0))


def host_prep(query, key, value, attn_mask, q_w, q_b, k_w, k_b, v_w, v_b, o_w, o_b):
    """Build (mode, in_maps) for the 8 cores."""
    mask = np.asarray(attn_mask).astype(bool)
    if np.array_equal(mask, np.triu(np.ones((S, S), bool), 1)):
        mode = "causal"
    elif not mask.any():
        mode = "nomask"
    else:
        mode = "generic"

    import ml_dtypes
    bf16 = ml_dtypes.bfloat16

    def img(xT, kc=KC):
        """[kc*128, W] -> SBUF image [128, kc*W] (kc-major per partition)."""
        r, w = xT.shape
        return np.ascontiguousarray(
            xT.reshape(kc, P, w).transpose(1, 0, 2).reshape(P, kc * w))

    def prep_x(x):
        xT = np.ascontiguousarray(np.asarray(x).T).astype(bf16)
        return np.concatenate(
            [img(xT[:, c0:c0 + cw]) for c0, cw in XGRP], axis=1)

    xs = {}
    for b in range(2):
        xs[b] = (prep_x(query[b]), prep_x(key[b]), prep_x(value[b]))

    tri = _tri_bias()
    biasT = None
    if mode == "generic":
        biasT = np.ascontiguousarray(
            np.where(mask, np.float32(NEG), np.float32(0.0)).T)
    has_bias = any(np.asarray(v).any() for v in (q_b, k_b, v_b))

    def prep_w(w, sl):
        return img(np.ascontiguousarray(np.asarray(w)[sl].T).astype(bf16))

    in_maps = []
    for c in range(8):
        b, g = divmod(c, 4)
        sl = slice(M_DIM * g, M_DIM * (g + 1))
        m = {
            "xqP": xs[b][0], "xkP": xs[b][1], "xvP": xs[b][2],
            "wqP": prep_w(q_w, sl),
            "wkP": prep_w(k_w, sl),
            "wvP": prep_w(v_w, sl),
            "owP": img(np.ascontiguousarray(
                np.asarray(o_w)[:, sl].T).astype(bf16), kc=2),
            "btri": tri,
        }
        if has_bias:
            m["wbias"] = np.stack([
                np.asarray(v)[sl].astype(bf16) for v in (q_b, k_b, v_b)])
        if mode == "generic":
            m["biasT"] = biasT
        in_maps.append(m)
    return mode, in_maps


def kernel(**inputs) -> np.ndarray:
    global LAST_RESULTS
    from concourse.bass_utils import run_bass_kernel_spmd

    mode, in_maps = host_prep(**inputs)
    has_bias = any(
        np.asarray(inputs[k]).any() for k in ("q_b", "k_b", "v_b"))
    nc = _get_nc(mode, has_bias)
    res = run_bass_kernel_spmd(nc, in_maps, core_ids=list(range(8)), trace=TRACE)
    LAST_RESULTS = res
    parts = [np.asarray(res.results[c]["out"]).astype(np.float32)
             for c in range(8)]
    o_b = np.asarray(inputs["o_b"]).astype(np.float32)
    out = np.stack([
        parts[0] + parts[1] + parts[2] + parts[3],
        parts[4] + parts[5] + parts[6] + parts[7],
    ], axis=0) + o_b[None, None, :]
    return out.astype(np.float32)
